# revision 12
# baseline (speedup 1.0000x reference)
"""Trainium2 Bass kernel for nn_ClassLayer_56564719289025.

Reference computation:  y = mean(|W|) * (x @ sign(W).T)
  x: [8192, 4096] f32, W: [4096, 4096] f32 -> y: [8192, 4096] f32

Strategy (8 NeuronCores):
  - Data-parallel over x rows: each core computes a 1024-row shard of y.
  - All matmuls run in fp8e4 DoubleRow mode (2 MACs/cell/cycle, measured
    ~2.9x bf16 per FLOP on this hardware). DoubleRow contracts pairs:
    out[m,n] = sum_k w[k,0,m]*s[k,0,n] + w[k,1,m]*s[k,1,n].
  - Accuracy: straight e4m3 quantization of x costs ~2.7% L2 error
    (gate is 2e-2), so the contraction is split:
      * k in [0, 1792): "straight" pairs — slots hold two different
        k-rows of e4m3(x); sign rows distinct.  (fast, fp8 precision)
      * k in [1792, 4096): "hi/lo" pairs — slot0 = e4m3(x), slot1 =
        e4m3(x - slot0), sign row broadcast into both slots via a
        stride-0 AP dim.  (full precision, half pair-throughput)
    Measured error 1.75e-2.
  - scale = mean(|W|) is computed on-device from a per-core 512-column
    bf16 slice of W^T plus a 512-byte AllReduce.
  - y is returned as bf16 (upcast on host) to halve output traffic.
  - Host prep is layout-only + elementwise casts: e4m3 casts of x and
    sign(W), packed so that every device DMA is a single contiguous
    block (x shard: one 6.55MB chunk; each sign o-block: one 2.1MB
    chunk).

Packed layouts:
  xPP [3200, 2048]: row (i*128+p), col (j*1024+m) = xpk[i*256+j*128+p, m]
    where xpk is the [6400, 1024] pair-expanded k-major x shard
    (straight region rows = plain k rows; hi/lo region packs (hi, lo)
    in the two 128-row halves of each 256 block).
  SB [8, 4096, 512]: [b, k, n] = sign(W)[b*512+n, k]  (o-block major,
    NOT pair-expanded; straight matmuls slice two k-tiles, hi/lo
    matmuls broadcast one k-tile into both pair slots).
"""

import numpy as np

import concourse.bacc as bacc
import concourse.bass_isa as bass_isa
import concourse.mybir as mybir
import concourse.tile as tile
from concourse.bass_utils import run_bass_kernel_spmd

TOKENS, D_IN, D_OUT, N_CORES = 8192, 4096, 4096, 8
P = 128            # SBUF partitions / DoubleRow half-contraction tile
OB = 512           # output-column block (one PSUM bank at fp32)
R_SHARD = TOKENS // N_CORES   # 1024 rows per core
RT = R_SHARD // P             # 8 row tiles per core
NB = D_OUT // OB              # 8 o-blocks
KT = D_IN // P                # 32 sign k-tiles per o-block

K_STRAIGHT = 1792             # straight-pair region (14 x 128)
N_STR = K_STRAIGHT // 256     # 7 straight DoubleRow matmuls
N_HILO = (D_IN - K_STRAIGHT) // P   # 18 hi/lo DoubleRow matmuls
NMM = N_STR + N_HILO          # 25 matmuls per (r, o-block) tile
KP = NMM * 256                # 6400 packed x rows

SCAN_W = D_OUT // N_CORES     # 512-column scan slice per core
INV_N = 1.0 / (D_IN * D_OUT)  # exactly 2**-24

bf16 = mybir.dt.bfloat16
fp8 = mybir.dt.float8e4
fp32 = mybir.dt.float32
DR = mybir.MatmulPerfMode.DoubleRow


def _emit(tc, xPP, SPB, wS, y, part, red, reps=1):
    nc = tc.nc
    xP4 = xPP.rearrange("(i p) (j m) -> p i j m", p=P, j=2)  # [128,25,2,1024]
    SP5 = SPB.rearrange("b (i p) (j n) -> b p i j n", p=P, j=2)  # [8,128,25,2,512]
    wS3 = wS.rearrange("(ko p) o -> p ko o", p=P)   # [128, 32, 512]
    y3 = y.rearrange("(rt p) o -> p rt o", p=P)     # [128, 8, 4096]

    with (
        tc.tile_pool(name="xpool", bufs=2) as xpool,
        tc.tile_pool(name="spool", bufs=2) as spool,
        tc.tile_pool(name="wscan", bufs=2) as wscan,
        tc.tile_pool(name="scpool", bufs=1) as scpool,
        tc.tile_pool(name="ypool", bufs=12) as ypool,
        tc.tile_pool(name="psum", bufs=8, space="PSUM") as psum,
    ):
        for _ in range(reps):
            # --- x shard and block-0 signs loaded i-sliced, interleaved so
            # the i-outer block-0 matmuls can start after the first pair ---
            # --- scale = mean(|W|) first: its DMA + AllReduce resolve
            # while block 0 computes, so evictions never queue on it ---
            acc = scpool.tile([P, 32], fp32, tag="acc")

            def scan_j(j):
                ws_t = wscan.tile([P, 8, OB], bf16, tag="ws")
                nc.sync.dma_start(ws_t[:], wS3[:, j * 8:(j + 1) * 8, :])
                nc.vector.tensor_reduce(
                    acc[:, j * 8:(j + 1) * 8], ws_t[:],
                    axis=mybir.AxisListType.X, op=mybir.AluOpType.add,
                    apply_absolute_value=True,
                )

            for j in range(4):
                scan_j(j)

            acc1 = scpool.tile([P, 1], fp32, tag="acc1")
            nc.vector.tensor_reduce(
                acc1[:], acc[:], axis=mybir.AxisListType.X, op=mybir.AluOpType.add
            )
            accs = scpool.tile([P, 1], fp32, tag="accs")
            nc.vector.tensor_scalar_mul(accs[:], acc1[:], INV_N)
            par_t = scpool.tile([P, 1], fp32, tag="par")
            nc.gpsimd.partition_all_reduce(
                par_t[:], accs[:], channels=P, reduce_op=bass_isa.ReduceOp.add
            )
            nc.sync.dma_start(part[:], par_t[:])
            nc.gpsimd.collective_compute(
                "AllReduce", mybir.AluOpType.add,
                [list(range(N_CORES))], [part[:]], [red[:]],
            )
            scale_sb = scpool.tile([P, 1], fp32, tag="scale")
            nc.sync.dma_start(scale_sb[:], red[:])

            x_sb = xpool.tile([P, NMM, 2, 1024], fp8, tag="x")
            S0 = spool.tile([P, NMM, 2, OB], fp8, tag="S")

            for i in range(NMM):
                nc.sync.dma_start(x_sb[:, i, :, :], xP4[:, i, :, :])
                nc.sync.dma_start(S0[:, i, :, :], SP5[0, :, i, :, :])

            def evict(ps, r, b):
                # two-step eviction: DVE copy frees the PSUM bank without
                # waiting on scale; the scale multiply binds later
                y32 = ypool.tile([P, OB], fp32, tag="y32")
                nc.vector.tensor_copy(out=y32[:], in_=ps[:])
                y16 = ypool.tile([P, OB], bf16, tag="y16")
                nc.scalar.mul(y16[:], y32[:], scale_sb[:])
                nc.sync.dma_start(y3[:, r, b * OB:(b + 1) * OB], y16[:])

            def mm(ps, S_b, i, r):
                nc.tensor.matmul(
                    ps[:],
                    lhsT=x_sb[:, i, :, r * P:(r + 1) * P],
                    rhs=S_b[:, i, :, :],
                    start=(i == 0),
                    stop=(i == NMM - 1),
                    perf_mode=DR,
                )

            # --- block 0: i-outer over 8 concurrent PSUM banks, so the PE
            # starts on the first (x[i], S0[i]) pair and tracks DMA supply ---
            ps0 = [
                psum.tile([P, OB], fp32, tag="ps", name=f"ps0_{r}")
                for r in range(RT)
            ]
            for i in range(NMM):
                for r in range(RT):
                    mm(ps0[r], S0, i, r)
            for r in range(RT):
                evict(ps0[r], r, 0)

            # --- blocks 1..7: r-inner, i-accumulate per group ---
            for b in range(1, NB):
                S_b = spool.tile([P, NMM, 2, OB], fp8, tag="S", name=f"S_{b}")
                nc.sync.dma_start(S_b[:], SP5[b, :, :, :, :])
                for r in range(RT):
                    ps = psum.tile([P, OB], fp32, tag="ps", name=f"ps_{b}_{r}")
                    for i in range(NMM):
                        mm(ps, S_b, i, r)
                    evict(ps, r, b)


def build(reps=1):
    nc = bacc.Bacc(
        "TRN2", target_bir_lowering=False, debug=False, num_devices=N_CORES
    )
    xPP = nc.dram_tensor("xPP", [NMM * P, 2048], fp8, kind="ExternalInput").ap()
    SPB = nc.dram_tensor("SPB", [NB, KP // 2, 1024], fp8, kind="ExternalInput").ap()
    wS = nc.dram_tensor("wscan", [D_IN, SCAN_W], bf16, kind="ExternalInput").ap()
    y = nc.dram_tensor("y", [R_SHARD, D_OUT], bf16, kind="ExternalOutput").ap()
    part = nc.dram_tensor("part", [P, 1], fp32, kind="Internal").ap()
    red = nc.dram_tensor("red", [P, 1], fp32, kind="Internal", addr_space="Shared").ap()

    with tile.TileContext(nc) as tc:
        _emit(tc, xPP, SPB, wS, y, part, red, reps=reps)
    nc.compile()
    return nc


_NC_CACHE = {}


def _get_nc(reps=1):
    if reps not in _NC_CACHE:
        _NC_CACHE[reps] = build(reps)
    return _NC_CACHE[reps]


def _pack_pairs(a_k):
    """[6400, W] pair-expanded k-major -> [3200, 2*W] DoubleRow-packed.

    Row (i*128+p), col (j*W+m) = a_k[i*256 + j*128 + p, m].
    """
    kp, w = a_k.shape
    a4 = a_k.reshape(kp // 256, 2, P, w)          # [i, j, p, w]
    return np.ascontiguousarray(
        a4.transpose(0, 2, 1, 3).reshape(kp // 2, 2 * w)
    )


def _make_in_maps(x, weight):
    import ml_dtypes

    f8 = np.dtype(mybir.dt.np(fp8))
    xT = np.ascontiguousarray(np.asarray(x, dtype=np.float32).T)  # [4096, 8192]
    wf = np.asarray(weight, dtype=np.float32)

    # pair-expanded k-major x: [6400, 8192]
    xpk = np.empty((KP, TOKENS), f8)
    xpk[:K_STRAIGHT] = xT[:K_STRAIGHT].astype(f8)
    xh = xT[K_STRAIGHT:]                                          # [2304, 8192]
    hi = xh.astype(f8)
    lo = (xh - hi.astype(np.float32)).astype(f8)
    hl = np.stack([hi.reshape(N_HILO, P, TOKENS),
                   lo.reshape(N_HILO, P, TOKENS)], axis=1)        # [18,2,128,T]
    xpk[K_STRAIGHT:] = hl.reshape(KP - K_STRAIGHT, TOKENS)

    # pair-expanded k-major signs [6400, 4096], hi/lo rows duplicated,
    # then packed per o-block: [8, 3200, 1024] (replicated per core)
    ST8 = np.sign(wf).T.astype(f8)                                # [4096, 4096]
    spk = np.empty((KP, D_OUT), f8)
    spk[:K_STRAIGHT] = ST8[:K_STRAIGHT]
    st3 = ST8[K_STRAIGHT:].reshape(N_HILO, P, D_OUT)
    spk[K_STRAIGHT:] = np.stack([st3, st3], axis=1).reshape(
        KP - K_STRAIGHT, D_OUT)
    SB_host = np.empty((NB, KP // 2, 1024), f8)
    for b in range(NB):
        SB_host[b] = _pack_pairs(spk[:, b * OB:(b + 1) * OB])

    wTb16 = wf.T.astype(ml_dtypes.bfloat16)

    in_maps = []
    for c in range(N_CORES):
        xsh = np.ascontiguousarray(xpk[:, c * R_SHARD:(c + 1) * R_SHARD])
        in_maps.append({
            "xPP": _pack_pairs(xsh),
            "SPB": SB_host,
            "wscan": np.ascontiguousarray(wTb16[:, c * SCAN_W:(c + 1) * SCAN_W]),
        })
    return in_maps


def kernel(x, weight):
    x = np.asarray(x)
    weight = np.asarray(weight)
    assert x.shape == (TOKENS, D_IN), x.shape
    assert weight.shape == (D_OUT, D_IN), weight.shape
    in_maps = _make_in_maps(x, weight)
    nc = _get_nc(1)
    last_exc = None
    for attempt in range(3):
        try:
            res = run_bass_kernel_spmd(nc, in_maps, core_ids=list(range(N_CORES)))
            break
        except Exception as e:  # transient NRT device errors — retry
            last_exc = e
            import time as _time

            _time.sleep(2.0 * (attempt + 1))
    else:
        raise last_exc
    return np.concatenate(
        [res.results[c]["y"] for c in range(N_CORES)], axis=0
    ).astype(np.float32)


# revision 13
# speedup vs baseline: 1.1448x; 1.1448x over previous
"""Trainium2 Bass kernel for nn_ClassLayer_56564719289025.

Reference computation:  y = mean(|W|) * (x @ sign(W).T)
  x: [8192, 4096] f32, W: [4096, 4096] f32 -> y: [8192, 4096] f32

Strategy (8 NeuronCores):
  - Data-parallel over x rows: each core computes a 1024-row shard of y.
  - All matmuls run in fp8e4 DoubleRow mode (2 MACs/cell/cycle, measured
    ~2.9x bf16 per FLOP on this hardware). DoubleRow contracts pairs:
    out[m,n] = sum_k w[k,0,m]*s[k,0,n] + w[k,1,m]*s[k,1,n].
  - Accuracy: straight e4m3 quantization of x costs ~2.7% L2 error
    (gate is 2e-2), so the contraction is split:
      * k in [0, 1792): "straight" pairs — slots hold two different
        k-rows of e4m3(x); sign rows distinct.  (fast, fp8 precision)
      * k in [1792, 4096): "hi/lo" pairs — slot0 = e4m3(x), slot1 =
        e4m3(x - slot0), sign row broadcast into both slots via a
        stride-0 AP dim.  (full precision, half pair-throughput)
    Measured error 1.75e-2.
  - scale = mean(|W|) is computed on-device from a per-core 512-column
    bf16 slice of W^T plus a 512-byte AllReduce.
  - y is returned as bf16 (upcast on host) to halve output traffic.
  - Host prep is layout-only + elementwise casts: e4m3 casts of x and
    sign(W), packed so that every device DMA is a single contiguous
    block (x shard: one 6.55MB chunk; each sign o-block: one 2.1MB
    chunk).

Packed layouts:
  xPP [3200, 2048]: row (i*128+p), col (j*1024+m) = xpk[i*256+j*128+p, m]
    where xpk is the [6400, 1024] pair-expanded k-major x shard
    (straight region rows = plain k rows; hi/lo region packs (hi, lo)
    in the two 128-row halves of each 256 block).
  SB [8, 4096, 512]: [b, k, n] = sign(W)[b*512+n, k]  (o-block major,
    NOT pair-expanded; straight matmuls slice two k-tiles, hi/lo
    matmuls broadcast one k-tile into both pair slots).
"""

import numpy as np

import concourse.bacc as bacc
import concourse.bass_isa as bass_isa
import concourse.mybir as mybir
import concourse.tile as tile
from concourse.bass_utils import run_bass_kernel_spmd

TOKENS, D_IN, D_OUT, N_CORES = 8192, 4096, 4096, 8
P = 128            # SBUF partitions / DoubleRow half-contraction tile
OB = 512           # output-column block (one PSUM bank at fp32)
R_SHARD = TOKENS // N_CORES   # 1024 rows per core
RT = R_SHARD // P             # 8 row tiles per core
NB = D_OUT // OB              # 8 o-blocks
KT = D_IN // P                # 32 sign k-tiles per o-block

K_STRAIGHT = 1792             # straight-pair region (14 x 128)
N_STR = K_STRAIGHT // 256     # 7 straight DoubleRow matmuls
N_HILO = (D_IN - K_STRAIGHT) // P   # 18 hi/lo DoubleRow matmuls
NMM = N_STR + N_HILO          # 25 matmuls per (r, o-block) tile
KP = NMM * 256                # 6400 packed x rows

SCAN_W = D_OUT // N_CORES     # 512-column scan slice per core
INV_N = 1.0 / (D_IN * D_OUT)  # exactly 2**-24

bf16 = mybir.dt.bfloat16
fp8 = mybir.dt.float8e4
fp32 = mybir.dt.float32
DR = mybir.MatmulPerfMode.DoubleRow


def _emit(tc, xPP, SPB, wS, y, part, red, reps=1):
    nc = tc.nc
    xP4 = xPP.rearrange("(i p) (j m) -> p i j m", p=P, j=2)  # [128,25,2,1024]
    SP5 = SPB.rearrange("b (i p) (j n) -> b p i j n", p=P, j=2)  # [8,128,25,2,512]
    wS3 = wS.rearrange("(ko p) o -> p ko o", p=P)   # [128, 32, 512]
    y3 = y.rearrange("(rt p) o -> p rt o", p=P)     # [128, 8, 4096]

    with (
        tc.tile_pool(name="xpool", bufs=2) as xpool,
        tc.tile_pool(name="spool", bufs=2) as spool,
        tc.tile_pool(name="wscan", bufs=2) as wscan,
        tc.tile_pool(name="scpool", bufs=1) as scpool,
        tc.tile_pool(name="ypool", bufs=12) as ypool,
        tc.tile_pool(name="psum", bufs=8, space="PSUM") as psum,
    ):
        for _ in range(reps):
            # --- x shard and block-0 signs loaded i-sliced, interleaved so
            # the i-outer block-0 matmuls can start after the first pair ---
            # --- x shard and block-0 signs loaded i-sliced, interleaved so
            # the i-outer block-0 matmuls can start after the first pair ---
            x_sb = xpool.tile([P, NMM, 2, 1024], fp8, tag="x")
            S0 = spool.tile([P, NMM, 2, OB], fp8, tag="S")

            for i in range(NMM):
                nc.sync.dma_start(x_sb[:, i, :, :], xP4[:, i, :, :])
                nc.sync.dma_start(S0[:, i, :, :], SP5[0, :, i, :, :])

            # --- scale = mean(|W|): abs-sum of the bf16 scan slice; the
            # multiply runs on ACT so a late scale never blocks DVE ---
            acc = scpool.tile([P, 32], fp32, tag="acc")

            def scan_j(j):
                ws_t = wscan.tile([P, 8, OB], bf16, tag="ws")
                nc.sync.dma_start(ws_t[:], wS3[:, j * 8:(j + 1) * 8, :])
                nc.vector.tensor_reduce(
                    acc[:, j * 8:(j + 1) * 8], ws_t[:],
                    axis=mybir.AxisListType.X, op=mybir.AluOpType.add,
                    apply_absolute_value=True,
                )

            for j in range(4):
                scan_j(j)

            acc1 = scpool.tile([P, 1], fp32, tag="acc1")
            nc.vector.tensor_reduce(
                acc1[:], acc[:], axis=mybir.AxisListType.X, op=mybir.AluOpType.add
            )
            accs = scpool.tile([P, 1], fp32, tag="accs")
            nc.vector.tensor_scalar_mul(accs[:], acc1[:], INV_N)
            par_t = scpool.tile([P, 1], fp32, tag="par")
            nc.gpsimd.partition_all_reduce(
                par_t[:], accs[:], channels=P, reduce_op=bass_isa.ReduceOp.add
            )
            nc.sync.dma_start(part[:], par_t[:])
            nc.gpsimd.collective_compute(
                "AllReduce", mybir.AluOpType.add,
                [list(range(N_CORES))], [part[:]], [red[:]],
            )
            scale_sb = scpool.tile([P, 1], fp32, tag="scale")
            nc.sync.dma_start(scale_sb[:], red[:])

            def evict(ps, r, b):
                # two-step eviction: DVE copy frees the PSUM bank without
                # waiting on scale; the scale multiply binds later
                y32 = ypool.tile([P, OB], fp32, tag="y32")
                nc.vector.tensor_copy(out=y32[:], in_=ps[:])
                y16 = ypool.tile([P, OB], bf16, tag="y16")
                nc.scalar.mul(y16[:], y32[:], scale_sb[:])
                nc.sync.dma_start(y3[:, r, b * OB:(b + 1) * OB], y16[:])

            def mm(ps, S_b, i, r):
                nc.tensor.matmul(
                    ps[:],
                    lhsT=x_sb[:, i, :, r * P:(r + 1) * P],
                    rhs=S_b[:, i, :, :],
                    start=(i == 0),
                    stop=(i == NMM - 1),
                    perf_mode=DR,
                )

            # --- block 0: i-outer over 8 concurrent PSUM banks, so the PE
            # starts on the first (x[i], S0[i]) pair and tracks DMA supply ---
            ps0 = [
                psum.tile([P, OB], fp32, tag="ps", name=f"ps0_{r}")
                for r in range(RT)
            ]
            for i in range(NMM):
                for r in range(RT):
                    mm(ps0[r], S0, i, r)
            for r in range(RT):
                evict(ps0[r], r, 0)

            # --- blocks 1..7: r-inner, i-accumulate per group ---
            for b in range(1, NB):
                S_b = spool.tile([P, NMM, 2, OB], fp8, tag="S", name=f"S_{b}")
                nc.sync.dma_start(S_b[:], SP5[b, :, :, :, :])
                for r in range(RT):
                    ps = psum.tile([P, OB], fp32, tag="ps", name=f"ps_{b}_{r}")
                    for i in range(NMM):
                        mm(ps, S_b, i, r)
                    evict(ps, r, b)


def build(reps=1):
    nc = bacc.Bacc(
        "TRN2", target_bir_lowering=False, debug=False, num_devices=N_CORES
    )
    xPP = nc.dram_tensor("xPP", [NMM * P, 2048], fp8, kind="ExternalInput").ap()
    SPB = nc.dram_tensor("SPB", [NB, KP // 2, 1024], fp8, kind="ExternalInput").ap()
    wS = nc.dram_tensor("wscan", [D_IN, SCAN_W], bf16, kind="ExternalInput").ap()
    y = nc.dram_tensor("y", [R_SHARD, D_OUT], bf16, kind="ExternalOutput").ap()
    part = nc.dram_tensor("part", [P, 1], fp32, kind="Internal").ap()
    red = nc.dram_tensor("red", [P, 1], fp32, kind="Internal", addr_space="Shared").ap()

    with tile.TileContext(nc) as tc:
        _emit(tc, xPP, SPB, wS, y, part, red, reps=reps)
    nc.compile()
    return nc


_NC_CACHE = {}


def _get_nc(reps=1):
    if reps not in _NC_CACHE:
        _NC_CACHE[reps] = build(reps)
    return _NC_CACHE[reps]


def _pack_pairs(a_k):
    """[6400, W] pair-expanded k-major -> [3200, 2*W] DoubleRow-packed.

    Row (i*128+p), col (j*W+m) = a_k[i*256 + j*128 + p, m].
    """
    kp, w = a_k.shape
    a4 = a_k.reshape(kp // 256, 2, P, w)          # [i, j, p, w]
    return np.ascontiguousarray(
        a4.transpose(0, 2, 1, 3).reshape(kp // 2, 2 * w)
    )


def _make_in_maps(x, weight):
    import ml_dtypes

    f8 = np.dtype(mybir.dt.np(fp8))
    xT = np.ascontiguousarray(np.asarray(x, dtype=np.float32).T)  # [4096, 8192]
    wf = np.asarray(weight, dtype=np.float32)

    # pair-expanded k-major x: [6400, 8192]
    xpk = np.empty((KP, TOKENS), f8)
    xpk[:K_STRAIGHT] = xT[:K_STRAIGHT].astype(f8)
    xh = xT[K_STRAIGHT:]                                          # [2304, 8192]
    hi = xh.astype(f8)
    lo = (xh - hi.astype(np.float32)).astype(f8)
    hl = np.stack([hi.reshape(N_HILO, P, TOKENS),
                   lo.reshape(N_HILO, P, TOKENS)], axis=1)        # [18,2,128,T]
    xpk[K_STRAIGHT:] = hl.reshape(KP - K_STRAIGHT, TOKENS)

    # pair-expanded k-major signs [6400, 4096], hi/lo rows duplicated,
    # then packed per o-block: [8, 3200, 1024] (replicated per core)
    ST8 = np.sign(wf).T.astype(f8)                                # [4096, 4096]
    spk = np.empty((KP, D_OUT), f8)
    spk[:K_STRAIGHT] = ST8[:K_STRAIGHT]
    st3 = ST8[K_STRAIGHT:].reshape(N_HILO, P, D_OUT)
    spk[K_STRAIGHT:] = np.stack([st3, st3], axis=1).reshape(
        KP - K_STRAIGHT, D_OUT)
    SB_host = np.empty((NB, KP // 2, 1024), f8)
    for b in range(NB):
        SB_host[b] = _pack_pairs(spk[:, b * OB:(b + 1) * OB])

    wTb16 = wf.T.astype(ml_dtypes.bfloat16)

    in_maps = []
    for c in range(N_CORES):
        xsh = np.ascontiguousarray(xpk[:, c * R_SHARD:(c + 1) * R_SHARD])
        in_maps.append({
            "xPP": _pack_pairs(xsh),
            "SPB": SB_host,
            "wscan": np.ascontiguousarray(wTb16[:, c * SCAN_W:(c + 1) * SCAN_W]),
        })
    return in_maps


def kernel(x, weight):
    x = np.asarray(x)
    weight = np.asarray(weight)
    assert x.shape == (TOKENS, D_IN), x.shape
    assert weight.shape == (D_OUT, D_IN), weight.shape
    in_maps = _make_in_maps(x, weight)
    nc = _get_nc(1)
    last_exc = None
    for attempt in range(3):
        try:
            res = run_bass_kernel_spmd(nc, in_maps, core_ids=list(range(N_CORES)))
            break
        except Exception as e:  # transient NRT device errors — retry
            last_exc = e
            import time as _time

            _time.sleep(2.0 * (attempt + 1))
    else:
        raise last_exc
    return np.concatenate(
        [res.results[c]["y"] for c in range(N_CORES)], axis=0
    ).astype(np.float32)
